# revision 15
# baseline (speedup 1.0000x reference)
"""Trainium2 Bass kernel for GaussianKernelGCNLayer.

Reference computation (per instance b of 2048 = 8*256):
  wf[b,k,d] = sum_n w[b,n,k] * f[b,n,d]         (n=32 neighbors, k=8 kernels)
  out[b,k,o] = sum_d wf[b,k,d] * CW[k,d,o]      (d=4096, o=512)

Sharding: data-parallel over the 2048 instances -> 256 per core on 8 cores.

Per-core device algorithm (all matmul inputs bf16, fp32 PSUM accumulate):
  Phase 1: for each group g of 4 instances, the 4*32=128 stacked neighbor
    features form the contraction dim.  lhsT = feature d-chunk [128, 128]
    (stationary, FWL), rhs = host-prebuilt block-diag weight tile
    [128, (k,bi)=32].  psum [128(d), NCH, k, bi] -> one DVE copy per group
    into the persistent wfT [128, NCH, NK, NGRP, 4] (d on partitions =
    exactly the phase-2 lhsT layout, no on-chip transpose).
  Phase 2: for each kernel k: out[b, k*512:+512] = wf_k @ CW_k as 32
    accumulating matmuls over d-chunks per m-tile; lhsT = wfT slice
    ([128 d, 128 b] contiguous -> FWL), rhs = CW chunk [128 d, 512 o].

All DRAM layouts are partition-major so every DMA moves >=8KB contiguous
per partition line: 73 dma_starts per iteration instead of 400.
"""

import sys

import numpy as np

try:
    import ml_dtypes
except ImportError:  # pragma: no cover
    ml_dtypes = None

for _p in ("/opt/trn_rl_repo",):
    if _p not in sys.path:
        sys.path.insert(0, _p)

NB, NI, NN, DIN = 8, 256, 32, 4096
NK, DKO = 8, 512
NCORES = 8
BL = NB * NI // NCORES  # 256 instances per core
NGRP = BL // 4          # 64 groups of 4 instances
NCH = DIN // 128        # 32 d-chunks
G = 2                   # groups per feature DMA (2MB transfers)
CQ = 16                 # d-chunks per conv-weight DMA (2MB transfers)
BF16 = ml_dtypes.bfloat16 if ml_dtypes is not None else None

_cached_nc = {}


def _build(repeat=1):
    from contextlib import ExitStack

    import concourse.bass as bass  # noqa: F401
    import concourse.tile as tile
    from concourse import bacc, mybir

    nc = bacc.Bacc(
        "TRN2",
        target_bir_lowering=False,
        debug=False,
        num_devices=NCORES,
    )

    f_d = nc.dram_tensor(
        "fcat", [128, NGRP, NCH, 128], mybir.dt.bfloat16, kind="ExternalInput"
    ).ap()
    wb_d = nc.dram_tensor(
        "wblk", [128, NGRP, NK, 4], mybir.dt.bfloat16, kind="ExternalInput"
    ).ap()
    cw_d = nc.dram_tensor(
        "cw", [128, NK, NCH, DKO], mybir.dt.bfloat16, kind="ExternalInput"
    ).ap()
    out_d = nc.dram_tensor(
        "out", [2, 128, NK, DKO], mybir.dt.bfloat16, kind="ExternalOutput"
    ).ap()

    ET = mybir.EngineType
    with ExitStack() as ctx:
        tc = ctx.enter_context(tile.TileContext(nc))
        const_pool = ctx.enter_context(tc.tile_pool(name="const", bufs=1))
        wbpool = ctx.enter_context(tc.tile_pool(name="wbpool", bufs=1))
        # fs (phase 1) and cwt (phase 2) share slots: same tag, disjoint
        # phases -> 3-deep prefetch for both within the SBUF budget.
        stream = ctx.enter_context(tc.tile_pool(name="stream", bufs=3))
        opool = ctx.enter_context(tc.tile_pool(name="opool", bufs=2))
        ps1 = ctx.enter_context(tc.tile_pool(name="ps1", bufs=2, space="PSUM"))
        ps2 = ctx.enter_context(tc.tile_pool(name="ps2", bufs=4, space="PSUM"))

        # Persistent transposed wf: [128 (d%128), chunk, k, g, bi] bf16.
        # For phase 2, wfT[:, c, k, mt*32:(mt+1)*32, :] is a contiguous
        # [128, 128] block -> FWL-eligible weight loads.
        wfT = const_pool.tile(
            [128, NCH, NK, NGRP, 4], mybir.dt.bfloat16, name="wfT"
        )

        def body():
            wb = wbpool.tile([128, NGRP, NK, 4], mybir.dt.bfloat16, name="wb")
            nc.sync.dma_start(wb[:], wb_d[:])

            # ---- Phase 1: wfT[d, (k,bi)] per instance-group ----
            for t in range(NGRP // G):
                fs = stream.tile(
                    [128, G, NCH, 128], mybir.dt.bfloat16, name="fs",
                    tag="stream",
                )
                eng = nc.sync if t % 2 == 0 else nc.scalar
                eng.dma_start(fs[:], f_d[:, t * G : (t + 1) * G])
                for gg in range(G):
                    g = t * G + gg
                    pt = ps1.tile(
                        [128, NCH, NK, 4], mybir.dt.float32, name="pt"
                    )
                    for c in range(NCH):
                        nc.tensor.matmul(
                            pt[:, c],
                            fs[:, gg, c],
                            wb[:, g],
                            start=True,
                            stop=True,
                        )
                    nc.vector.tensor_copy(wfT[:, :, :, g, :], pt[:])

            # ---- Phase 2: out = wf @ CW, k-outer, both m-tiles ----
            ots = None
            for k in range(NK):
                po0 = ps2.tile(
                    [128, DKO], mybir.dt.float32, name="po0", tag="po"
                )
                po1 = ps2.tile(
                    [128, DKO], mybir.dt.float32, name="po1", tag="po"
                )
                pos = (po0, po1)
                for q in range(NCH // CQ):
                    cwt = stream.tile(
                        [128, CQ, DKO], mybir.dt.bfloat16, name="cwt",
                        tag="stream",
                    )
                    nc.sync.dma_start(cwt[:], cw_d[:, k, q * CQ : (q + 1) * CQ])
                    # mt outer: 16 consecutive matmuls accumulate into the
                    # same PSUM bank (no per-MM bank alternation).
                    for mt in range(2):
                        for cc in range(CQ):
                            c = q * CQ + cc
                            nc.tensor.matmul(
                                pos[mt][:],
                                wfT[:, c, k, mt * 32 : (mt + 1) * 32, :],
                                cwt[:, cc],
                                start=(c == 0),
                                stop=(c == NCH - 1),
                            )
                if k % 2 == 0:
                    ots = [
                        opool.tile(
                            [128, 2, DKO], mybir.dt.bfloat16, name=f"ot{mt}",
                            tag=f"ot{mt}",
                        )
                        for mt in range(2)
                    ]
                for mt in range(2):
                    nc.vector.tensor_copy(ots[mt][:, k % 2, :], pos[mt][:])
                if k % 2 == 1:
                    for mt in range(2):
                        nc.scalar.dma_start(
                            out_d[mt, :, k - 1 : k + 1, :], ots[mt][:]
                        )

        if repeat > 1:
            # Unroll 2 iterations per hardware-loop trip: halves the
            # all-engine back-edge barriers and lets iteration B's feature
            # DMAs prefetch behind iteration A's phase-2 compute.
            assert repeat % 2 == 0
            ctx.enter_context(
                tc.For_i(
                    0,
                    repeat // 2,
                    1,
                    hint_engines=(ET.PE, ET.SP, ET.DVE, ET.Activation, ET.Pool),
                )
            )
            body()
            body()
        else:
            body()

    nc.compile()
    return nc


def _prep_inputs(neighbourhood_features, neighbourhood_weights, conv_weight):
    f = np.asarray(neighbourhood_features, dtype=np.float32).reshape(
        NB * NI, NN, DIN
    )
    w = np.asarray(neighbourhood_weights, dtype=np.float32).reshape(
        NB * NI, NN, NK
    )
    cw = np.asarray(conv_weight, dtype=np.float32)
    # [128, NK, NCH, DKO]: cwT[p, k, c, o] = cw[k, c*128+p, o]
    cwT = np.ascontiguousarray(
        cw.reshape(NK, NCH, 128, DKO).transpose(2, 0, 1, 3)
    ).astype(BF16)
    in_maps = []
    for i in range(NCORES):
        fl = f[i * BL : (i + 1) * BL].reshape(NGRP, 128, DIN)
        # [128, NGRP, NCH, 128], partition-major
        fcat = np.ascontiguousarray(fl.transpose(1, 0, 2)).astype(BF16).reshape(
            128, NGRP, NCH, 128
        )
        wl = w[i * BL : (i + 1) * BL].reshape(NGRP, 4, NN, NK)
        wblk = np.zeros((NGRP, 128, NK, 4), dtype=np.float32)
        for bi in range(4):
            wblk[:, bi * 32 : (bi + 1) * 32, :, bi] = wl[:, bi]
        wbT = np.ascontiguousarray(wblk.transpose(1, 0, 2, 3)).astype(BF16)
        in_maps.append({"fcat": fcat, "wblk": wbT, "cw": cwT})
    return in_maps


def _execute(neighbourhood_features, neighbourhood_weights, conv_weight, trace=False):
    global _cached_nc
    if 1 not in _cached_nc:
        _cached_nc[1] = _build()
    nc = _cached_nc[1]
    from concourse import bass_utils

    in_maps = _prep_inputs(
        neighbourhood_features, neighbourhood_weights, conv_weight
    )
    res = bass_utils.run_bass_kernel_spmd(
        nc, in_maps, core_ids=list(range(NCORES)), trace=trace
    )
    outs = [
        np.asarray(res.results[i]["out"], dtype=np.float32).reshape(BL, NK * DKO)
        for i in range(NCORES)
    ]
    full = np.concatenate(outs, axis=0)
    return full.reshape(NB, NI, NK * DKO), res


def kernel(neighbourhood_features, neighbourhood_weights, conv_weight):
    out, _ = _execute(
        neighbourhood_features, neighbourhood_weights, conv_weight, trace=False
    )
    return out
